# revision 10
# baseline (speedup 1.0000x reference)
"""Multi-head self-attention (RoPE) Trainium2 Bass kernel.

Shards batch (B=8) across 8 NeuronCores, one batch element per core.
Instruction-count-minimized design: the backend charges ~100us per
instruction per engine with engines running in parallel, so the kernel
keeps the tensor engine (384 matmuls, the f16 floor) as the only busy
engine: RoPE runs as 7 wide DVE ops over a [128, 8192] tile, exp and
PSUM-drain copies run on ACT, softmax normalization is one divide per
head, and denominators come free from ones-columns in the augmented V.
"""
import os
import sys

# The kernel needs the 8 axon-tunneled NeuronCores visible to jax; a
# JAX_PLATFORMS=cpu pin (used by some harnesses for the reference) would
# hide them. Clear it before jax initializes through the concourse imports.
os.environ.pop("JAX_PLATFORMS", None)

sys.path.insert(0, "/opt/trn_rl_repo")

_REPS = int(os.environ.get("KREPS", "1"))
_PH = int(os.environ.get("KPHASES", "4"))  # 1=QKV 2=+scores 3=+AV 4=full

import numpy as np
from contextlib import ExitStack

import concourse.bass as bass
import concourse.tile as tile
from concourse import bacc, mybir

f32 = mybir.dt.float32
f16 = mybir.dt.float16
AF = mybir.ActivationFunctionType
ALU = mybir.AluOpType

B, L, DIM = 8, 1024, 512
NH, HD = 8, 64
SCALE = HD ** -0.5
NCORES = 8


def _build_nc():
    nc = bacc.Bacc("TRN2", target_bir_lowering=False, debug=False, enable_asserts=False)

    xT = nc.dram_tensor("xT", (DIM, L), f16, kind="ExternalInput")
    wq = nc.dram_tensor("wq", (DIM, 2 * DIM), f16, kind="ExternalInput")  # Q|K cols
    wv = nc.dram_tensor("wv", (DIM, DIM), f16, kind="ExternalInput")      # V cols
    wp = nc.dram_tensor("wp", (DIM, DIM), f16, kind="ExternalInput")
    cosT = nc.dram_tensor("cosT", (128, L), f16, kind="ExternalInput")
    sinT = nc.dram_tensor("sinT", (128, L), f16, kind="ExternalInput")
    bias = nc.dram_tensor("bias", (128, 8), f32, kind="ExternalInput")
    y = nc.dram_tensor("y", (L, DIM), f32, kind="ExternalOutput")

    with ExitStack() as ctx:
        tc = ctx.enter_context(tile.TileContext(nc))
        cst = ctx.enter_context(tc.tile_pool(name="cst", bufs=1))
        sc = ctx.enter_context(tc.tile_pool(name="sc", bufs=1))
        pTp = ctx.enter_context(tc.tile_pool(name="pTp", bufs=12))
        ysb = ctx.enter_context(tc.tile_pool(name="ysb", bufs=1))

        # ---- load inputs (once) ----
        xT_all = cst.tile([128, 4 * L], f16, name="t", tag="xTall")
        wq_all = cst.tile([128, 4 * 2 * DIM], f16, name="t", tag="wqall")
        wv_all = cst.tile([128, 4 * DIM], f16, name="t", tag="wvall")
        wp_all = cst.tile([128, 4 * DIM], f16, name="t", tag="wpall")
        for big, dram, w in ((xT_all, xT, L), (wq_all, wq, 2 * DIM),
                             (wv_all, wv, DIM), (wp_all, wp, DIM)):
            nc.sync.dma_start(
                big[:].rearrange("p (kc w) -> p kc w", kc=4),
                dram[:].rearrange("(kc p) w -> p kc w", p=128))
        xT_sb = [xT_all[:, i * L:(i + 1) * L] for i in range(4)]
        wq_sb = [wq_all[:, i * 2 * DIM:(i + 1) * 2 * DIM] for i in range(4)]
        wv_sb = [wv_all[:, i * DIM:(i + 1) * DIM] for i in range(4)]
        wp_sb = [wp_all[:, i * DIM:(i + 1) * DIM] for i in range(4)]
        cos_sb = cst.tile([128, L], f16, name="t", tag="cos")
        sin_sb = cst.tile([128, L], f16, name="t", tag="sin")
        bias_sb = cst.tile([128, 8], f32, name="t", tag="bias")
        nc.sync.dma_start(cos_sb[:], cosT[:])
        nc.sync.dma_start(sin_sb[:], sinT[:])
        nc.sync.dma_start(bias_sb[:], bias[:])

        # persistent working tiles
        qraw = cst.tile([128, 8 * L], f16, name="t", tag="qraw")
        qsw = cst.tile([128, 8 * L], f16, name="t", tag="qsw")
        # qkTp: 16 blocks of [128, L]; block 2m+two holds one head's 64 rows
        # (rows 0:64 for even heads, 64:128 for odd), other half stays zero so
        # every scores matmul is a full K=128 contraction (K<128 and nonzero
        # partition offsets are ~10x slower per instruction on this backend).
        qkTp = cst.tile([128, 16 * L], f16, name="t", tag="qkTp")
        # vaug[lb]: [128 keys, NH*128]; head h block: even h -> v in cols 0:64,
        # ones in 64:128; odd h -> ones in 0:64, v in 64:128 (so the attention
        # numerator lands on the partition rows the output layout needs).
        vaug = [cst.tile([128, NH * 128], f16, name="t", tag=f"vaug{i}") for i in range(8)]
        outT = [cst.tile([128, L], f16, name="t", tag=f"outT{c}") for c in range(4)]

        yall0 = ysb.tile([128, 8 * DIM], f32, name="t", tag="yall")
        nc.vector.memset(yall0[:], 0.0)
        nc.vector.memset(qkTp[:], 0.0)
        # ones columns of vaug never change: set once.
        for lb in range(8):
            v3 = vaug[lb][:].rearrange("p (h2 c) -> p h2 c", h2=4)
            nc.vector.memset(v3[:, :, 64:128], 1.0)   # even-head ones
            nc.vector.memset(v3[:, :, 128:192], 1.0)  # odd-head ones

        def emit_body(rep):
            # ---------- phase 1: QK projection into qraw ----------
            with tc.tile_pool(name=f"qkps{rep}", bufs=4, space="PSUM") as qk_ps:
                for m in range(8):
                    ps = qk_ps.tile([128, L], f32, name="t", tag="qkps")
                    for kc in range(4):
                        for qb in range(2):
                            nc.tensor.matmul(
                                ps[:, qb * 512:(qb + 1) * 512],
                                wq_sb[kc][:, m * 128:(m + 1) * 128],
                                xT_sb[kc][:, qb * 512:(qb + 1) * 512],
                                start=(kc == 0), stop=(kc == 3))
                    nc.scalar.copy(qraw[:, m * L:(m + 1) * L], ps[:])

                # ---------- wide RoPE over all 8 m-blocks ----------
                for (do, so) in ((0, 32), (32, 0), (64, 96), (96, 64)):
                    nc.vector.tensor_copy(qsw[do:do + 32, :], qraw[so:so + 32, :])
                for m in range(8):
                    nc.vector.tensor_mul(qraw[:, m * L:(m + 1) * L],
                                         qraw[:, m * L:(m + 1) * L], cos_sb[:])
                    nc.vector.tensor_mul(qsw[:, m * L:(m + 1) * L],
                                         qsw[:, m * L:(m + 1) * L], sin_sb[:])
                qkTp3 = qkTp[:].rearrange("p (m two w) -> p m two w", two=2, w=L)
                q1w3 = qraw[:].rearrange("p (m w) -> p m w", w=L)
                qsw3 = qsw[:].rearrange("p (m w) -> p m w", w=L)
                nc.vector.tensor_add(qkTp3[0:64, :, 0, :], q1w3[0:64], qsw3[0:64])
                nc.vector.tensor_add(qkTp3[64:128, :, 1, :], q1w3[64:128], qsw3[64:128])

            # ---------- phase 1b: V projection ----------
            with tc.tile_pool(name=f"vps{rep}", bufs=8, space="PSUM") as v_ps:
                for lb in range(8):
                    vps = v_ps.tile([128, DIM], f32, name="t", tag="vps")
                    for kc in range(4):
                        nc.tensor.matmul(
                            vps[:],
                            xT_sb[kc][:, lb * 128:(lb + 1) * 128],
                            wv_sb[kc][:],
                            start=(kc == 0), stop=(kc == 3))
                    v3 = vaug[lb][:].rearrange("p (h2 c) -> p h2 c", h2=4)
                    p3 = vps[:].rearrange("p (h2 c) -> p h2 c", h2=4)
                    nc.scalar.copy(v3[:, :, 0:64], p3[:, :, 0:64])      # even heads
                    nc.scalar.copy(v3[:, :, 192:256], p3[:, :, 64:128])  # odd heads

            # ---------- phase 2: attention, pipelined per head ----------
            with tc.tile_pool(name=f"sps{rep}", bufs=1, space="PSUM") as s_ps, \
                 tc.tile_pool(name=f"avps{rep}", bufs=2, space="PSUM") as av_ps:
                for h in range(NH if _PH >= 2 else 0):
                    qcol = h * L
                    kcol = (8 + h) * L
                    pts = []
                    for kb2 in range(4):
                        s = s_ps.tile([128, 2048], f32, name="t", tag="s")
                        for half in range(2):
                            kb = kb2 * 2 + half
                            for qb in range(2):
                                nc.tensor.matmul(
                                    s[:, half * 1024 + qb * 512:
                                       half * 1024 + (qb + 1) * 512],
                                    qkTp[:, kcol + kb * 128:kcol + (kb + 1) * 128],
                                    qkTp[:, qcol + qb * 512:qcol + (qb + 1) * 512],
                                    start=True, stop=True)
                        pt = pTp.tile([128, 2048], f16, name="t", tag="pT")
                        nc.scalar.activation(pt[:], s[:], AF.Exp,
                                             bias=bias_sb[:, 2 * kb2:2 * kb2 + 1],
                                             scale=SCALE)
                        pts.append(pt)

                    if _PH < 3:
                        continue
                    X = av_ps.tile([128, L], f32, name="t", tag="avX")
                    for kc in range(8):
                        pt = pts[kc // 2]
                        off = (kc % 2) * 1024
                        va = vaug[kc][:, h * 128:(h + 1) * 128]
                        for qb in range(2):
                            nc.tensor.matmul(
                                X[:, qb * 512:(qb + 1) * 512], va,
                                pt[:, off + qb * 512:off + (qb + 1) * 512],
                                start=(kc == 0), stop=(kc == 7))
                    # numerator rows match the output-layout rows; denominator
                    # rows are the complement. recip needs an SBUF-staged input
                    # (custom-DVE op misreads PSUM), hence the D copy.
                    c = h // 2
                    D = sc.tile([64, L], f32, name="t", tag="D")
                    R = sc.tile([64, L], f32, name="t", tag="R")
                    if h % 2 == 0:
                        nc.vector.tensor_copy(D[:], X[64:128, :])
                        nc.vector.reciprocal_approx_fast(R[:], D[:])
                        nc.vector.tensor_mul(outT[c][0:64, :], X[0:64, :], R[:])
                    else:
                        nc.vector.tensor_copy(D[:], X[0:64, :])
                        nc.vector.reciprocal_approx_fast(R[:], D[:])
                        nc.vector.tensor_mul(outT[c][64:128, :], X[64:128, :], R[:])

            # ---------- phase 3: output projection ----------
            with tc.tile_pool(name=f"yps{rep}", bufs=8, space="PSUM") as y_ps:
                yall = yall0
                for lb in range(8 if _PH >= 4 else 0):
                    yp = y_ps.tile([128, DIM], f32, name="t", tag="yps")
                    for c in range(4):
                        nc.tensor.matmul(
                            yp[:],
                            outT[c][:, lb * 128:(lb + 1) * 128],
                            wp_sb[c][:],
                            start=(c == 0), stop=(c == 3))
                    nc.scalar.copy(yall[:, lb * DIM:(lb + 1) * DIM], yp[:])
                nc.sync.dma_start(
                    y[:].rearrange("(lb p) d -> p lb d", p=128),
                    yall[:].rearrange("p (lb d) -> p lb d", lb=8))

        for rep in range(_REPS):
            emit_body(rep)

    nc.compile()
    return nc


def _rope_tables():
    inv_freq = 1.0 / (10000.0 ** (np.arange(0, HD, 2, dtype=np.float32) / HD))
    t = np.arange(L, dtype=np.float32)
    freqs = np.outer(t, inv_freq)                      # (L, 32)
    emb = np.concatenate([freqs, freqs], axis=-1)      # (L, 64)
    cos = np.cos(emb).T                                # (64, L)
    sin = np.sin(emb).T                                # (64, L)
    sign = np.where(np.arange(HD) < HD // 2, -1.0, 1.0)[:, None].astype(np.float32)
    sin_s = sin * sign
    cosT = np.tile(cos, (2, 1)).astype(np.float16)     # (128, L)
    sinT = np.tile(sin_s, (2, 1)).astype(np.float16)   # (128, L)
    return cosT, sinT


_NC = None


def _get_nc():
    global _NC
    if _NC is None:
        _NC = _build_nc()
    return _NC


def kernel(x, mask, w_qkv, w_proj):
    x = np.asarray(x, dtype=np.float32)
    mask = np.asarray(mask)
    w_qkv = np.asarray(w_qkv, dtype=np.float32)
    w_proj = np.asarray(w_proj, dtype=np.float32)

    nc = _get_nc()
    cosT, sinT = _rope_tables()

    wq = np.ascontiguousarray(w_qkv[:, :2 * DIM]).astype(np.float16)
    wv = np.ascontiguousarray(w_qkv[:, 2 * DIM:]).astype(np.float16)
    wp = w_proj.astype(np.float16)

    in_maps = []
    for b in range(NCORES):
        xTb = np.ascontiguousarray(x[b].T).astype(np.float16)      # (512, 1024)
        bias_b = np.where(mask[b].reshape(8, 128).T, 0.0, -1e9).astype(np.float32)
        in_maps.append({
            "xT": xTb, "wq": wq, "wv": wv, "wp": wp,
            "cosT": cosT, "sinT": sinT, "bias": bias_b,
        })

    from concourse.bass_utils import run_bass_kernel_spmd
    res = run_bass_kernel_spmd(nc, in_maps, core_ids=list(range(NCORES)))
    out = np.stack([res.results[c]["y"] for c in range(NCORES)], axis=0)
    return out.astype(np.float32)


# revision 11
# speedup vs baseline: 1.1724x; 1.1724x over previous
"""Multi-head self-attention (RoPE) Trainium2 Bass kernel.

Shards batch (B=8) across 8 NeuronCores, one batch element per core.
Instruction-count-minimized design: the backend charges ~100us per
instruction per engine with engines running in parallel, so the kernel
keeps the tensor engine (384 matmuls, the f16 floor) as the only busy
engine: RoPE runs as 7 wide DVE ops over a [128, 8192] tile, exp and
PSUM-drain copies run on ACT, softmax normalization is one divide per
head, and denominators come free from ones-columns in the augmented V.
"""
import os
import sys

# The kernel needs the 8 axon-tunneled NeuronCores visible to jax; a
# JAX_PLATFORMS=cpu pin (used by some harnesses for the reference) would
# hide them. Clear it before jax initializes through the concourse imports.
os.environ.pop("JAX_PLATFORMS", None)

sys.path.insert(0, "/opt/trn_rl_repo")

_REPS = int(os.environ.get("KREPS", "1"))
_PH = int(os.environ.get("KPHASES", "4"))  # 1=QKV 2=+scores 3=+AV 4=full
_NOEXP = int(os.environ.get("KNOEXP", "0"))  # 1: skip exp ACTs (breaks output; for cost attribution)

import numpy as np
from contextlib import ExitStack

import concourse.bass as bass
import concourse.tile as tile
from concourse import bacc, mybir

f32 = mybir.dt.float32
f16 = mybir.dt.float16
AF = mybir.ActivationFunctionType
ALU = mybir.AluOpType

B, L, DIM = 8, 1024, 512
NH, HD = 8, 64
SCALE = HD ** -0.5
NCORES = 8


def _build_nc():
    nc = bacc.Bacc("TRN2", target_bir_lowering=False, debug=False, enable_asserts=False)

    xT = nc.dram_tensor("xT", (DIM, L), f16, kind="ExternalInput")
    wq = nc.dram_tensor("wq", (DIM, 2 * DIM), f16, kind="ExternalInput")  # Q|K cols
    wv = nc.dram_tensor("wv", (DIM, DIM), f16, kind="ExternalInput")      # V cols
    wp = nc.dram_tensor("wp", (DIM, DIM), f16, kind="ExternalInput")
    cosT = nc.dram_tensor("cosT", (128, L), f16, kind="ExternalInput")
    sinT = nc.dram_tensor("sinT", (128, L), f16, kind="ExternalInput")
    bias = nc.dram_tensor("bias", (128, 8), f32, kind="ExternalInput")
    y = nc.dram_tensor("y", (L, DIM), f32, kind="ExternalOutput")

    with ExitStack() as ctx:
        tc = ctx.enter_context(tile.TileContext(nc))
        cst = ctx.enter_context(tc.tile_pool(name="cst", bufs=1))
        sc = ctx.enter_context(tc.tile_pool(name="sc", bufs=1))
        pTp = ctx.enter_context(tc.tile_pool(name="pTp", bufs=12))
        ysb = ctx.enter_context(tc.tile_pool(name="ysb", bufs=1))

        # ---- load inputs (once) ----
        xT_all = cst.tile([128, 4 * L], f16, name="t", tag="xTall")
        wq_all = cst.tile([128, 4 * 2 * DIM], f16, name="t", tag="wqall")
        wv_all = cst.tile([128, 4 * DIM], f16, name="t", tag="wvall")
        wp_all = cst.tile([128, 4 * DIM], f16, name="t", tag="wpall")
        for big, dram, w in ((xT_all, xT, L), (wq_all, wq, 2 * DIM),
                             (wv_all, wv, DIM), (wp_all, wp, DIM)):
            nc.sync.dma_start(
                big[:].rearrange("p (kc w) -> p kc w", kc=4),
                dram[:].rearrange("(kc p) w -> p kc w", p=128))
        xT_sb = [xT_all[:, i * L:(i + 1) * L] for i in range(4)]
        wq_sb = [wq_all[:, i * 2 * DIM:(i + 1) * 2 * DIM] for i in range(4)]
        wv_sb = [wv_all[:, i * DIM:(i + 1) * DIM] for i in range(4)]
        wp_sb = [wp_all[:, i * DIM:(i + 1) * DIM] for i in range(4)]
        cos_sb = cst.tile([128, L], f16, name="t", tag="cos")
        sin_sb = cst.tile([128, L], f16, name="t", tag="sin")
        bias_sb = cst.tile([128, 8], f32, name="t", tag="bias")
        nc.sync.dma_start(cos_sb[:], cosT[:])
        nc.sync.dma_start(sin_sb[:], sinT[:])
        nc.sync.dma_start(bias_sb[:], bias[:])

        # persistent working tiles
        qraw = cst.tile([128, 8 * L], f16, name="t", tag="qraw")
        qsw = cst.tile([128, 8 * L], f16, name="t", tag="qsw")
        # qkTp: 16 blocks of [128, L]; block 2m+two holds one head's 64 rows
        # (rows 0:64 for even heads, 64:128 for odd), other half stays zero so
        # every scores matmul is a full K=128 contraction (K<128 and nonzero
        # partition offsets are ~10x slower per instruction on this backend).
        qkTp = cst.tile([128, 16 * L], f16, name="t", tag="qkTp")
        # vaug[lb]: [128 keys, NH*128]; head h block: even h -> v in cols 0:64,
        # ones in 64:128; odd h -> ones in 0:64, v in 64:128 (so the attention
        # numerator lands on the partition rows the output layout needs).
        vaug = [cst.tile([128, NH * 128], f16, name="t", tag=f"vaug{i}") for i in range(8)]
        outT = [cst.tile([128, L], f16, name="t", tag=f"outT{c}") for c in range(4)]

        yall0 = ysb.tile([128, 8 * DIM], f32, name="t", tag="yall")
        nc.vector.memset(yall0[:], 0.0)
        nc.vector.memset(qkTp[:], 0.0)
        # ones columns of vaug never change: set once.
        for lb in range(8):
            v3 = vaug[lb][:].rearrange("p (h2 c) -> p h2 c", h2=4)
            nc.vector.memset(v3[:, :, 64:128], 1.0)   # even-head ones
            nc.vector.memset(v3[:, :, 128:192], 1.0)  # odd-head ones

        def emit_body(rep):
            # ---------- phase 1: QK projection into qraw ----------
            with tc.tile_pool(name=f"qkps{rep}", bufs=4, space="PSUM") as qk_ps:
                for m in range(8):
                    ps = qk_ps.tile([128, L], f32, name="t", tag="qkps")
                    for kc in range(4):
                        for qb in range(2):
                            nc.tensor.matmul(
                                ps[:, qb * 512:(qb + 1) * 512],
                                wq_sb[kc][:, m * 128:(m + 1) * 128],
                                xT_sb[kc][:, qb * 512:(qb + 1) * 512],
                                start=(kc == 0), stop=(kc == 3))
                    nc.scalar.copy(qraw[:, m * L:(m + 1) * L], ps[:])

                # ---------- wide RoPE over all 8 m-blocks ----------
                for (do, so) in ((0, 32), (32, 0), (64, 96), (96, 64)):
                    nc.vector.tensor_copy(qsw[do:do + 32, :], qraw[so:so + 32, :])
                for m in range(8):
                    nc.vector.tensor_mul(qraw[:, m * L:(m + 1) * L],
                                         qraw[:, m * L:(m + 1) * L], cos_sb[:])
                    nc.vector.tensor_mul(qsw[:, m * L:(m + 1) * L],
                                         qsw[:, m * L:(m + 1) * L], sin_sb[:])
                qkTp3 = qkTp[:].rearrange("p (m two w) -> p m two w", two=2, w=L)
                q1w3 = qraw[:].rearrange("p (m w) -> p m w", w=L)
                qsw3 = qsw[:].rearrange("p (m w) -> p m w", w=L)
                nc.vector.tensor_add(qkTp3[0:64, :, 0, :], q1w3[0:64], qsw3[0:64])
                nc.vector.tensor_add(qkTp3[64:128, :, 1, :], q1w3[64:128], qsw3[64:128])

            # ---------- phase 1b: V projection ----------
            with tc.tile_pool(name=f"vps{rep}", bufs=8, space="PSUM") as v_ps:
                for lb in range(8):
                    vps = v_ps.tile([128, DIM], f32, name="t", tag="vps")
                    for kc in range(4):
                        nc.tensor.matmul(
                            vps[:],
                            xT_sb[kc][:, lb * 128:(lb + 1) * 128],
                            wv_sb[kc][:],
                            start=(kc == 0), stop=(kc == 3))
                    v3 = vaug[lb][:].rearrange("p (h2 c) -> p h2 c", h2=4)
                    p3 = vps[:].rearrange("p (h2 c) -> p h2 c", h2=4)
                    nc.scalar.copy(v3[:, :, 0:64], p3[:, :, 0:64])      # even heads
                    nc.scalar.copy(v3[:, :, 192:256], p3[:, :, 64:128])  # odd heads

            # ---------- phase 2: attention, pipelined per head ----------
            with tc.tile_pool(name=f"sps{rep}", bufs=1, space="PSUM") as s_ps, \
                 tc.tile_pool(name=f"avps{rep}", bufs=2, space="PSUM") as av_ps:
                for h in range(NH if _PH >= 2 else 0):
                    qcol = h * L
                    kcol = (8 + h) * L
                    pts = []
                    for kb2 in range(4):
                        s = s_ps.tile([128, 2048], f32, name="t", tag="s")
                        for half in range(2):
                            kb = kb2 * 2 + half
                            for qb in range(2):
                                nc.tensor.matmul(
                                    s[:, half * 1024 + qb * 512:
                                       half * 1024 + (qb + 1) * 512],
                                    qkTp[:, kcol + kb * 128:kcol + (kb + 1) * 128],
                                    qkTp[:, qcol + qb * 512:qcol + (qb + 1) * 512],
                                    start=True, stop=True)
                        pt = pTp.tile([128, 2048], f16, name="t", tag="pT")
                        nc.scalar.activation(pt[:], s[:], AF.Exp,
                                             bias=bias_sb[:, 2 * kb2:2 * kb2 + 1],
                                             scale=SCALE)
                        pts.append(pt)

                    if _PH < 3:
                        continue
                    X = av_ps.tile([128, L], f32, name="t", tag="avX")
                    for kc in range(8):
                        pt = pts[kc // 2]
                        off = (kc % 2) * 1024
                        va = vaug[kc][:, h * 128:(h + 1) * 128]
                        for qb in range(2):
                            nc.tensor.matmul(
                                X[:, qb * 512:(qb + 1) * 512], va,
                                pt[:, off + qb * 512:off + (qb + 1) * 512],
                                start=(kc == 0), stop=(kc == 7))
                    # numerator rows match the output-layout rows; denominator
                    # rows are the complement. recip needs an SBUF-staged input
                    # (custom-DVE op misreads PSUM), hence the D copy.
                    c = h // 2
                    D = sc.tile([64, L], f32, name="t", tag="D")
                    R = sc.tile([64, L], f32, name="t", tag="R")
                    if h % 2 == 0:
                        nc.vector.tensor_copy(D[:], X[64:128, :])
                        nc.vector.reciprocal_approx_fast(R[:], D[:])
                        nc.vector.tensor_mul(outT[c][0:64, :], X[0:64, :], R[:])
                    else:
                        nc.vector.tensor_copy(D[:], X[0:64, :])
                        nc.vector.reciprocal_approx_fast(R[:], D[:])
                        nc.vector.tensor_mul(outT[c][64:128, :], X[64:128, :], R[:])

            # ---------- phase 3: output projection ----------
            with tc.tile_pool(name=f"yps{rep}", bufs=8, space="PSUM") as y_ps:
                yall = yall0
                for lb in range(8 if _PH >= 4 else 0):
                    yp = y_ps.tile([128, DIM], f32, name="t", tag="yps")
                    for c in range(4):
                        nc.tensor.matmul(
                            yp[:],
                            outT[c][:, lb * 128:(lb + 1) * 128],
                            wp_sb[c][:],
                            start=(c == 0), stop=(c == 3))
                    nc.scalar.copy(yall[:, lb * DIM:(lb + 1) * DIM], yp[:])
                nc.sync.dma_start(
                    y[:].rearrange("(lb p) d -> p lb d", p=128),
                    yall[:].rearrange("p (lb d) -> p lb d", lb=8))

        for rep in range(_REPS):
            emit_body(rep)

    nc.compile()
    return nc


def _rope_tables():
    inv_freq = 1.0 / (10000.0 ** (np.arange(0, HD, 2, dtype=np.float32) / HD))
    t = np.arange(L, dtype=np.float32)
    freqs = np.outer(t, inv_freq)                      # (L, 32)
    emb = np.concatenate([freqs, freqs], axis=-1)      # (L, 64)
    cos = np.cos(emb).T                                # (64, L)
    sin = np.sin(emb).T                                # (64, L)
    sign = np.where(np.arange(HD) < HD // 2, -1.0, 1.0)[:, None].astype(np.float32)
    sin_s = sin * sign
    cosT = np.tile(cos, (2, 1)).astype(np.float16)     # (128, L)
    sinT = np.tile(sin_s, (2, 1)).astype(np.float16)   # (128, L)
    return cosT, sinT


_NC = None


def _get_nc():
    global _NC
    if _NC is None:
        _NC = _build_nc()
    return _NC


def kernel(x, mask, w_qkv, w_proj):
    x = np.asarray(x, dtype=np.float32)
    mask = np.asarray(mask)
    w_qkv = np.asarray(w_qkv, dtype=np.float32)
    w_proj = np.asarray(w_proj, dtype=np.float32)

    nc = _get_nc()
    cosT, sinT = _rope_tables()

    wq = np.ascontiguousarray(w_qkv[:, :2 * DIM]).astype(np.float16)
    wv = np.ascontiguousarray(w_qkv[:, 2 * DIM:]).astype(np.float16)
    wp = w_proj.astype(np.float16)

    in_maps = []
    for b in range(NCORES):
        xTb = np.ascontiguousarray(x[b].T).astype(np.float16)      # (512, 1024)
        bias_b = np.where(mask[b].reshape(8, 128).T, 0.0, -1e9).astype(np.float32)
        in_maps.append({
            "xT": xTb, "wq": wq, "wv": wv, "wp": wp,
            "cosT": cosT, "sinT": sinT, "bias": bias_b,
        })

    from concourse.bass_utils import run_bass_kernel_spmd
    res = run_bass_kernel_spmd(nc, in_maps, core_ids=list(range(NCORES)))
    out = np.stack([res.results[c]["y"] for c in range(NCORES)], axis=0)
    return out.astype(np.float32)
